# revision 26
# baseline (speedup 1.0000x reference)
"""Trainium2 Bass kernel for a dense transformer block (pre-LN, MHA + MLP).

Sharding: 8 cores; core c handles batch b = c // 4, query block qb = c % 4
(512 tokens). Each core recomputes K/V for its batch's full 2048-token
sequence (zero cross-core communication — a measured 4-rank AllGather of
K/V costs ~90us wall, more than the ~80us of PE time it would save), then
runs attention for its 512 query tokens and the MLP on them.

Mixed precision, validated on hardware (4.5e-3 max rel err vs fp32 ref):
 - QKV / attn-out projections: fp8e4m3 weights (host-prescaled x16) and
   fp8 LN activations, with DoubleRow matmuls (contraction 256/instr, half
   the instructions and cycles); the PSUM descale folds into the ACT copy.
 - AV: fp8 softmax weights (exp writes fp8) x fp8 V, DoubleRow over
   kv-token pair blocks. Softmax/AV averaging washes out the fp8 error.
 - fc1/fc2: bf16 (fp8 here measures ~2.7e-2 max err — over the 2e-2 gate,
   since MLP matmul quantization error hits the output directly).
 - scores (q.k): f32r x f32r (full PE rate at free-dim >= 256); LN stats
   in fp32 via ones-row matmuls.

All activations are feature-major ([feature, token]); the host pre-
transposes x and all weights so every matmul is layout-natural. Softmax
runs without max-subtraction (|q.k|/8 <~ 5); denominators come from a
ones-row appended to V inside the AV matmul. LN's rstd = exp(-0.5*ln(v))
and Exp share one ACT table set (avoids 1.3us table reloads); softmax
reciprocals use the single-pass DVE approx (DVE's exact reciprocal
measures 3-4us per instruction on hardware).
"""
import ml_dtypes
import numpy as np

import concourse.bass as bass
import concourse.mybir as mybir
import concourse.tile as tile
from concourse import bacc
from concourse.bass_utils import run_bass_kernel_spmd

P = 128
C = 1024
NCT = C // P          # 8 feature tiles
TKV = 2048            # kv tokens per core (sequence length)
TQ = 512              # query tokens per core
HID = 4096
NHT = HID // P        # 32 hidden tiles
H = 16
HD = 64
NHP = H // 2          # 8 head pairs
CHUNK = 512           # kv tokens processed per pipeline chunk
NCHUNK = TKV // CHUNK # 4
NJCL = CHUNK // P     # 4 j-subchunks of 128 per chunk
EPS = 1e-5
SCALE = HD ** -0.5

f32 = mybir.dt.float32
f32r = mybir.dt.float32r
bf16 = mybir.dt.bfloat16
f8 = mybir.dt.float8e4
W8SCALE = 16.0
Act = mybir.ActivationFunctionType


def _emit_ln(nc, ones_stat, eps_t, ps_pool, sb_pool, x_of_ct, out, g_t, b_t,
             F, stat_tag="ln_stat", stat_bufs=1):
    """LayerNorm over the feature (partition) dim for one <=512-token chunk.

    x_of_ct(ct) -> [128, F] input AP; out: [128, NCT, F] tile; g_t/b_t:
    [128, NCT] per-feature scale/bias tiles.
    """
    ps_stat = ps_pool.tile([1, 2 * F], f32, tag=stat_tag, bufs=stat_bufs,
                           name="ps_stat")
    for ct in range(NCT):
        x_ct = x_of_ct(ct)
        sq = sb_pool.tile([P, F], f32r, tag="ln_sq", bufs=3)
        nc.vector.tensor_mul(sq[:], x_ct, x_ct)
        nc.tensor.matmul(ps_stat[:, 0:F], (ones_stat[:]), (x_ct),
                         start=(ct == 0), stop=(ct == NCT - 1))
        nc.tensor.matmul(ps_stat[:, F:2 * F], (ones_stat[:]), (sq[:]),
                         start=(ct == 0), stop=(ct == NCT - 1))
    mu = sb_pool.tile([1, F], f32, tag="ln_mu", bufs=1)
    var = sb_pool.tile([1, F], f32, tag="ln_var", bufs=1)
    nc.vector.tensor_scalar_mul(mu[:], ps_stat[:, 0:F], 1.0 / C)
    nc.vector.tensor_scalar_mul(var[:], ps_stat[:, F:2 * F], 1.0 / C)
    mu2 = sb_pool.tile([1, F], f32, tag="ln_mu2", bufs=1)
    nc.vector.tensor_mul(mu2[:], mu[:], mu[:])
    nc.vector.tensor_sub(var[:], var[:], mu2[:])
    # rstd = exp(-0.5*ln(var + eps)); Ln/Exp share the attention Exp's
    # ACT table set, so no ACT_TABLE_LOAD in the A-C phases.
    nc.scalar.activation(var[:], var[:], Act.Ln, bias=eps_t[0:1, :])
    nc.scalar.activation(var[:], var[:], Act.Exp, scale=-0.5)
    mu_b = sb_pool.tile([P, F], f32, tag="ln_mub", bufs=2)
    rstd_b = sb_pool.tile([P, F], f32, tag="ln_rstdb", bufs=2)
    nc.gpsimd.partition_broadcast(mu_b[:], mu[:])
    nc.gpsimd.partition_broadcast(rstd_b[:], var[:])
    for ct in range(NCT):
        o = out[:, ct, :]
        nc.vector.tensor_sub(o, x_of_ct(ct), mu_b[:])
        nc.vector.tensor_mul(o, o, rstd_b[:])
        nc.vector.tensor_scalar(o, o, g_t[:, ct:ct + 1], b_t[:, ct:ct + 1],
                                op0=mybir.AluOpType.mult,
                                op1=mybir.AluOpType.add)


def build_program(sim_standin=False):
    # CoreSim lacks Gelu; Tanh has identical ACT cost, so the sim variant
    # swaps it in for modeled-time runs (numerics then checked vs a matching
    # numpy reference).
    gelu_fn = Act.Tanh if sim_standin else Act.Gelu
    nc = bacc.Bacc()

    # DRAM I/O (per core). All feature-major / pre-transposed by the host.
    xkvT = nc.dram_tensor("xkvT", [C, TKV], f32r, kind="ExternalInput")
    xqT = nc.dram_tensor("xqT", [C, TQ], f32r, kind="ExternalInput")
    wqT = nc.dram_tensor("wqT", [C, C], f8, kind="ExternalInput")
    wkT = nc.dram_tensor("wkT", [C, C], f8, kind="ExternalInput")
    wvT = nc.dram_tensor("wvT", [C, C], f8, kind="ExternalInput")
    wpT = nc.dram_tensor("wpT", [C, C], f8, kind="ExternalInput")
    w1T = nc.dram_tensor("w1T", [C, HID], bf16, kind="ExternalInput")
    w2T = nc.dram_tensor("w2T", [HID, C], bf16, kind="ExternalInput")
    bp = nc.dram_tensor("bp", [C], f32, kind="ExternalInput")
    b1 = nc.dram_tensor("b1", [HID], f32, kind="ExternalInput")
    b2 = nc.dram_tensor("b2", [C], f32, kind="ExternalInput")
    ln1g = nc.dram_tensor("ln1g", [C], f32, kind="ExternalInput")
    ln1b = nc.dram_tensor("ln1b", [C], f32, kind="ExternalInput")
    ln2g = nc.dram_tensor("ln2g", [C], f32, kind="ExternalInput")
    ln2b = nc.dram_tensor("ln2b", [C], f32, kind="ExternalInput")
    outT = nc.dram_tensor("outT", [C, TQ], f32, kind="ExternalOutput")

    def pct(t, i):  # rows [i*128, (i+1)*128) of a DRAM matrix
        return t[i * P:(i + 1) * P, :]

    with tile.TileContext(nc) as tc:
      with (
          tc.tile_pool(name="const", bufs=1) as const,
          tc.tile_pool(name="px2", bufs=1) as px2,
      ):
        ones_f32 = const.tile([P, HD], f32)
        nc.vector.memset(ones_f32[:], 1.0)
        ones_stat = const.tile([P, 1], f32r)
        nc.vector.tensor_copy(ones_stat[:], ones_f32[:, 0:1])
        eps_t = const.tile([P, 1], f32)
        nc.vector.memset(eps_t[:], EPS)

        def vec_param(t, n, name):
            v = const.tile([P, n // P], f32, name=name)
            nc.sync.dma_start(v[:], t[:].rearrange("(ct p) -> p ct", p=P))
            return v

        x2T = px2.tile([P, NCT, TQ], f32r)  # attn residual (phases C-D)

        with tc.tile_pool(name="persist", bufs=1) as persist:
            qT = persist.tile([P, NHP, TQ], f32r)    # Q, feature-major
            attnT = persist.tile([P, NHP, TQ], f32) # AV accum (unnorm.)
            attn_bf = persist.tile([P, NHP, TQ], f8)
            # softmax denominators: head h -> partition 32*(h%4), slot h//4
            den = persist.tile([P, 4, TQ], f32)

            with (
                tc.tile_pool(name="pb_x", bufs=2) as pb_x,
                tc.tile_pool(name="pb_ln", bufs=3) as pb_ln,
                tc.tile_pool(name="pb_w", bufs=3) as pb_w,
                tc.tile_pool(name="pb_kv", bufs=1) as pb_kv,
                tc.tile_pool(name="pb_sb", bufs=2) as pb_sb,
                tc.tile_pool(name="pb_ps", bufs=1, space="PSUM") as pb_ps,
                tc.tile_pool(name="pb_psav", bufs=1, space="PSUM") as pb_psav,
            ):
                # ---- Phase A: LN1(xq) and Q projection ----
                xq_ln = pb_x.tile([P, NCT, TQ], f32r, tag="xkv",
                                  name="xq_ln")
                for ct in range(NCT):
                    nc.sync.dma_start(
                        xq_ln[:, ct, :],
                        xqT[ct * P:(ct + 1) * P, :])
                g1_t = vec_param(ln1g, C, "g1_t")
                bb1_t = vec_param(ln1b, C, "bb1_t")
                g2_t = vec_param(ln2g, C, "g2_t")
                bb2_t = vec_param(ln2b, C, "bb2_t")
                bp_t = vec_param(bp, C, "bp_t")
                b1_t = vec_param(b1, HID, "b1_t")
                b2_t = vec_param(b2, C, "b2_t")
                ln1q = pb_ln.tile([P, NCT, TQ], f8, tag="lnkv",
                                  name="ln1q")
                _emit_ln(nc, ones_stat, eps_t, pb_ps, pb_sb,
                         lambda ct: xq_ln[:, ct, :], ln1q, g1_t, bb1_t, TQ,
                         stat_tag="sc_ps", stat_bufs=2)
                DR = mybir.MatmulPerfMode.DoubleRow
                NKT = C // 256  # 4 k-tiles for the fp8 DoubleRow projections
                for hp in range(NHP):
                    wq_t = pb_w.tile([P, NKT, 2, P], f8, tag="wk",
                                     name="wq_t")
                    nc.sync.dma_start(
                        wq_t[:],
                        wqT[:, hp * P:(hp + 1) * P].rearrange(
                            "(kt j p) m -> p kt j m", p=P, j=2))
                    ps = pb_ps.tile([P, TQ], f32, tag="kv_ps", bufs=2,
                                    name="q_ps")
                    for kt in range(NKT):
                        nc.tensor.matmul(
                            ps[:], wq_t[:, kt], ln1q[:, 2 * kt:2 * kt + 2, :],
                            start=(kt == 0), stop=(kt == NKT - 1),
                            perf_mode=DR)
                    nc.scalar.activation(qT[:, hp, :], ps[:], Act.Identity,
                                         scale=1.0 / W8SCALE)

                # ---- Phase B: pipelined chunks ----
                # kT/v are fixed tiles; chunk ch+1's K/V projections are
                # interleaved per-head into chunk ch's attention loop, so
                # Tile's region-level WAR deps let next-chunk PE work fill
                # the ACT-bound attention window.
                kT_c = pb_kv.tile([P, NHP, CHUNK], f32r, tag="kT")
                v_c = pb_kv.tile([P, NJCL, H, HD + 1], f8, tag="v")
                nc.vector.tensor_copy(
                    v_c[:, :, :, HD],
                    ones_f32[:].rearrange("p (a b) -> p a b", a=NJCL))

                def emit_ln_chunk(ch):
                    j0 = ch * CHUNK
                    xkv_t = pb_x.tile([P, NCT, CHUNK], f32r, tag="xkv")
                    nc.sync.dma_start(
                        xkv_t[:],
                        xkvT[:, j0:j0 + CHUNK].rearrange(
                            "(ct p) f -> p ct f", p=P))
                    lnkv = pb_ln.tile([P, NCT, CHUNK], f8, tag="lnkv")
                    _emit_ln(nc, ones_stat, eps_t, pb_ps, pb_sb,
                             lambda ct: xkv_t[:, ct, :], lnkv, g1_t, bb1_t,
                             CHUNK, stat_tag="sc_ps", stat_bufs=2)
                    return lnkv

                def emit_k(lnkv, hp):
                    wk_t = pb_w.tile([P, NKT, 2, P], f8, tag="wk")
                    nc.sync.dma_start(
                        wk_t[:],
                        wkT[:, hp * P:(hp + 1) * P].rearrange(
                            "(kt j p) m -> p kt j m", p=P, j=2))
                    ps = pb_ps.tile([P, CHUNK], f32, tag="kv_ps", bufs=2,
                                    name="k_ps")
                    for kt in range(NKT):
                        nc.tensor.matmul(
                            ps[:], wk_t[:, kt], lnkv[:, 2 * kt:2 * kt + 2, :],
                            start=(kt == 0), stop=(kt == NKT - 1),
                            perf_mode=DR)
                    nc.scalar.activation(kT_c[:, hp, :], ps[:], Act.Identity,
                                         scale=1.0 / W8SCALE)

                def emit_v(lnkv, half):
                    wv_t = pb_w.tile([P, NKT, 2, 512], f8, tag="wv", bufs=1)
                    nc.sync.dma_start(
                        wv_t[:],
                        wvT[:, half * 512:(half + 1) * 512].rearrange(
                            "(kt j p) f -> p kt j f", p=P, j=2))
                    for jl in range(NJCL):
                        ps = pb_ps.tile([P, CHUNK], f32, tag="kv_ps",
                                        bufs=2, name="v_ps")
                        for kt in range(NKT):
                            nc.tensor.matmul(
                                ps[:],
                                lnkv[:, 2 * kt:2 * kt + 2, jl * P:(jl + 1) * P],
                                wv_t[:, kt],
                                start=(kt == 0), stop=(kt == NKT - 1),
                                perf_mode=DR)
                        nc.scalar.activation(
                            v_c[:, jl, half * 8:(half + 1) * 8, 0:HD],
                            ps[:].rearrange("p (h d) -> p h d", d=HD),
                            Act.Identity, scale=1.0 / W8SCALE)

                def emit_attn(ch, hp):
                    last = ch == NCHUNK - 1
                    e_all = pb_kv.tile([P, NJCL, 2 * TQ], f8, tag="e_all",
                                       bufs=3)
                    ps_av0 = pb_psav.tile([HD + 1, TQ], f32, tag="av0",
                                          name="ps_av0")
                    ps_av1 = pb_psav.tile([HD + 1, TQ], f32, tag="av1",
                                          name="ps_av1")
                    for jl in range(NJCL):
                        ps_sc = pb_ps.tile([P, 2 * TQ], f32, tag="sc_ps",
                                           bufs=2, name="ps_sc")
                        nc.tensor.matmul(
                            ps_sc[:, 0:TQ],
                            kT_c[0:HD, hp, jl * P:(jl + 1) * P],
                            qT[0:HD, hp, :], start=True, stop=True)
                        nc.tensor.matmul(
                            ps_sc[:, TQ:2 * TQ],
                            kT_c[HD:P, hp, jl * P:(jl + 1) * P],
                            qT[HD:P, hp, :], start=True, stop=True)
                        nc.scalar.activation(e_all[:, jl, :], ps_sc[:],
                                             Act.Exp, scale=SCALE)
                    # AV in fp8 DoubleRow over kv-token pair blocks
                    for jp in range(NJCL // 2):
                        nc.tensor.matmul(
                            ps_av0[:], v_c[:, 2 * jp:2 * jp + 2, 2 * hp, :],
                            e_all[:, 2 * jp:2 * jp + 2, 0:TQ],
                            start=(jp == 0), stop=(jp == NJCL // 2 - 1),
                            perf_mode=DR)
                        nc.tensor.matmul(
                            ps_av1[:],
                            v_c[:, 2 * jp:2 * jp + 2, 2 * hp + 1, :],
                            e_all[:, 2 * jp:2 * jp + 2, TQ:2 * TQ],
                            start=(jp == 0), stop=(jp == NJCL // 2 - 1),
                            perf_mode=DR)
                    # accumulate into attnT / den; on the last chunk,
                    # normalize this head pair in place
                    for i, ps_av in ((0, ps_av0), (1, ps_av1)):
                        h = 2 * hp + i
                        a_dst = attnT[i * HD:(i + 1) * HD, hp, :]
                        dp = 32 * (h % 4)
                        d_dst = den[dp:dp + 1, h // 4, :]
                        if ch == 0:
                            nc.vector.tensor_copy(a_dst, ps_av[0:HD, :])
                            nc.vector.tensor_copy(d_dst, ps_av[HD:HD + 1, :])
                        else:
                            nc.vector.tensor_add(a_dst, a_dst, ps_av[0:HD, :])
                            nc.vector.tensor_add(d_dst, d_dst,
                                                 ps_av[HD:HD + 1, :])
                        if last:
                            d_stage = pb_sb.tile([1, TQ], f32,
                                                 tag="d_stage", bufs=2)
                            nc.vector.reciprocal_approx_fast(d_stage[:],
                                                             d_dst)
                            rcp_b = pb_sb.tile([P, TQ], f32, tag="rcp_b",
                                               bufs=2)
                            nc.gpsimd.partition_broadcast(rcp_b[:],
                                                          d_stage[:])
                            nc.vector.tensor_mul(
                                attn_bf[i * HD:(i + 1) * HD, hp, :], a_dst,
                                rcp_b[i * HD:(i + 1) * HD, :])

                lnkv_t = {0: emit_ln_chunk(0)}
                for hp in range(NHP):
                    emit_k(lnkv_t[0], hp)
                emit_v(lnkv_t[0], 0)
                emit_v(lnkv_t[0], 1)
                lnkv_t[1] = emit_ln_chunk(1)
                for ch in range(NCHUNK):
                    for hp in range(NHP):
                        emit_attn(ch, hp)
                        if ch + 1 < NCHUNK:
                            emit_k(lnkv_t[ch + 1], hp)
                    if ch + 1 < NCHUNK:
                        emit_v(lnkv_t[ch + 1], 0)
                        emit_v(lnkv_t[ch + 1], 1)
                    if ch + 2 < NCHUNK:
                        lnkv_t[ch + 2] = emit_ln_chunk(ch + 2)

                # ---- Phase C (same pools): projection + residual ----
                xq2 = pb_x.tile([P, NCT, TQ], f32r, tag="xkv", name="xq2")
                nc.sync.dma_start(
                    xq2[:], xqT[:].rearrange("(ct p) f -> p ct f", p=P))
                for ct in range(NCT):
                    wp_t = pb_w.tile([P, NKT, 2, P], f8, tag="wk",
                                     name="wp_t")
                    nc.sync.dma_start(
                        wp_t[:],
                        wpT[:, ct * P:(ct + 1) * P].rearrange(
                            "(kt j p) m -> p kt j m", p=P, j=2))
                    ps = pb_ps.tile([P, TQ], f32, tag="kv_ps", bufs=2,
                                    name="proj_ps")
                    for kt in range(NKT):
                        nc.tensor.matmul(
                            ps[:], wp_t[:, kt],
                            attn_bf[:, 2 * kt:2 * kt + 2, :],
                            start=(kt == 0), stop=(kt == NKT - 1),
                            perf_mode=DR)
                    o = x2T[:, ct, :]
                    nc.scalar.activation(o, ps[:], Act.Identity,
                                         bias=bp_t[:, ct:ct + 1],
                                         scale=1.0 / W8SCALE)
                    nc.vector.tensor_add(o, o, xq2[:, ct, :])

        # ---- Phase D: LN2, fc1+gelu, fc2 + residual ----
        with (
            tc.tile_pool(name="pd_sb", bufs=3) as pd_sb,
            tc.tile_pool(name="pd_ln", bufs=1) as pd_ln,
            tc.tile_pool(name="pd_g", bufs=1) as pd_g,
            tc.tile_pool(name="pd_w", bufs=3) as pd_w,
            tc.tile_pool(name="pd_ps", bufs=2, space="PSUM") as pd_ps,
            tc.tile_pool(name="pd_ps2", bufs=1, space="PSUM") as pd_ps2,
        ):
            ln2T = pd_ln.tile([P, NCT, TQ], bf16)
            _emit_ln(nc, ones_stat, eps_t, pd_ps, pd_sb,
                     lambda ct: x2T[:, ct, :], ln2T, g2_t, bb2_t, TQ)

            g1T = pd_g.tile([P, NHT, TQ], bf16)
            for htg in range(NHT // 4):
                w1_t = pd_w.tile([P, NCT, 512], bf16, tag="w1")
                nc.sync.dma_start(
                    w1_t[:],
                    w1T[:, htg * 512:(htg + 1) * 512].rearrange(
                        "(ct p) f -> p ct f", p=P))
                for hl in range(4):
                    ht = htg * 4 + hl
                    ps = pd_ps.tile([P, TQ], f32, tag="fc1_ps")
                    for ct in range(NCT):
                        nc.tensor.matmul(
                            ps[:], w1_t[:, ct, hl * P:(hl + 1) * P],
                            ln2T[:, ct, :],
                            start=(ct == 0), stop=(ct == NCT - 1))
                    nc.scalar.activation(g1T[:, ht, :], ps[:], gelu_fn,
                                         bias=b1_t[:, ht:ht + 1])

            for ctg in range(2):
                ps_out = [pd_ps2.tile([P, TQ], f32, tag=f"fc2_{i}",
                                      name=f"fc2_ps_{i}")
                          for i in range(4)]
                for htg4 in range(NHT // 4):
                    w2_t = pd_w.tile([P, 4, 512], bf16, tag="w2", bufs=3)
                    nc.sync.dma_start(
                        w2_t[:],
                        w2T[htg4 * 512:(htg4 + 1) * 512,
                            ctg * 512:(ctg + 1) * 512].rearrange(
                            "(h p) f -> p h f", p=P))
                    for hl in range(4):
                        ht = htg4 * 4 + hl
                        for cl in range(4):
                            nc.tensor.matmul(
                                ps_out[cl][:],
                                w2_t[:, hl, cl * P:(cl + 1) * P],
                                g1T[:, ht, :],
                                start=(ht == 0), stop=(ht == NHT - 1))
                for cl in range(4):
                    ct = ctg * 4 + cl
                    o = pd_sb.tile([P, TQ], f32, tag="out_t")
                    nc.scalar.activation(o[:], ps_out[cl][:], Act.Identity,
                                         bias=b2_t[:, ct:ct + 1])
                    nc.vector.tensor_add(o[:], o[:], x2T[:, ct, :])
                    nc.sync.dma_start(outT[ct * P:(ct + 1) * P, :], o[:])

    nc.finalize()
    return nc


_program = None


def _get_program():
    global _program
    if _program is None:
        _program = build_program()
    return _program


def kernel(**inputs):
    x = np.asarray(inputs["x"], dtype=np.float32)
    B, N, _ = x.shape  # [2, 2048, 1024]

    def T(a):
        return np.ascontiguousarray(np.asarray(a, dtype=np.float32).T)

    def Tb(a):
        return np.ascontiguousarray(
            np.asarray(a, dtype=np.float32).T.astype(ml_dtypes.bfloat16))

    def T8(a):
        return np.ascontiguousarray(
            (np.asarray(a, dtype=np.float32).T * W8SCALE)
            .astype(ml_dtypes.float8_e4m3))

    w_qkv = np.asarray(inputs["w_qkv"], dtype=np.float32)
    shared = {
        "wqT": T8(w_qkv[0:C]),
        "wkT": T8(w_qkv[C:2 * C]),
        "wvT": T8(w_qkv[2 * C:3 * C]),
        "wpT": T8(inputs["w_proj"]),
        "w1T": Tb(inputs["w_fc1"]),
        "w2T": Tb(inputs["w_fc2"]),
        "bp": np.asarray(inputs["b_proj"], dtype=np.float32),
        "b1": np.asarray(inputs["b_fc1"], dtype=np.float32),
        "b2": np.asarray(inputs["b_fc2"], dtype=np.float32),
        "ln1g": np.asarray(inputs["ln1_g"], dtype=np.float32),
        "ln1b": np.asarray(inputs["ln1_b"], dtype=np.float32),
        "ln2g": np.asarray(inputs["ln2_g"], dtype=np.float32),
        "ln2b": np.asarray(inputs["ln2_b"], dtype=np.float32),
    }
    xT = [T(x[b]) for b in range(B)]  # [C, N] each
    in_maps = []
    for core in range(8):
        b, qb = core // 4, core % 4
        m = dict(shared)
        m["xkvT"] = xT[b]
        m["xqT"] = np.ascontiguousarray(xT[b][:, qb * TQ:(qb + 1) * TQ])
        in_maps.append(m)

    nc = _get_program()
    res = run_bass_kernel_spmd(nc, in_maps, list(range(8)))

    out = np.empty((B, N, C), dtype=np.float32)
    for core in range(8):
        b, qb = core // 4, core % 4
        out[b, qb * TQ:(qb + 1) * TQ, :] = res.results[core]["outT"].T
    return out

